# revision 14
# baseline (speedup 1.0000x reference)
"""Sparse-attention Bass kernel for 8 TRN2 NeuronCores — v3.

Sharding: core c owns batch b = c//4 and query rows [1024*(c%4), +1024) of
that batch (NQ=1024). Each core computes K^T/V for its OWN batch element
only (half the redundant QKV work of the 2-batch query sharding).

Per-core pipeline (engines balanced: PE ~527us, ACT ~492us, DVE ~494us):
  - batT is streamed per 512-token column block (bc tiles); K^T chunks, V
    tiles and Q^T are produced by deferred dense matmuls interleaved into
    the attention iterations so the PE never idles.
  - attention loop over (head-pair hh, key block jb):
      S^T via 4 K=32 matmuls (tile_position row bands) -> st0/st1 PSUM
      exp: ACT activation(Exp) -> bf16 e tiles; every 5th tile1 instead
           uses a DVE Schraudolph exp (tensor_scalar a*s+b -> int16 bits
           reinterpreted as bf16) to offload the saturated ACT engine
      mask: DVE tensor_mul with the row-block mask tile
      U^T/rowsum: PE matmuls [V|1]^T @ E^T accumulated in av PSUM
  - group end: av evacuated to SBUF with one DVE copy (frees the PSUM
    bank for the next head pair), then rowsum reciprocal + gpsimd
    partition broadcast + normalize multiply run off-PSUM, deferred into
    the next group's early iterations.
  - out = pre^T-packed matmul vs w_proj, contiguous DMA out.
"""

import math
import numpy as np
from contextlib import ExitStack

import concourse.bass as bass
import concourse.tile as tile
from concourse import bacc, mybir
from concourse.alu_op_type import AluOpType
from concourse.bass_utils import run_bass_kernel_spmd

BF16 = mybir.dt.bfloat16
F32 = mybir.dt.float32
I16 = mybir.dt.int16
NPBF16 = mybir.dt.np(BF16)

B, N, DIM, H, D = 2, 4096, 512, 16, 32
NCORES = 8
NQ = 1024                   # query rows per core (quarter of one batch elem)
JB = N // 128               # key blocks (32)
JC = N // 512               # batT column chunks (8)
SCALE = float(D) ** -0.5
# Schraudolph exp constants: bf16 bits of e^x ~= int16((x*log2e + 127)*128)
SCH_A = SCALE * math.log2(math.e) * 128.0
SCH_B = 127.0 * 128.0

_CACHE = {}


def build_nc():
    nc = bacc.Bacc("TRN2", target_bir_lowering=False, debug=False)

    batT = nc.declare_dram_parameter("batt", [DIM, N], BF16, isOutput=False)
    qrT = nc.declare_dram_parameter("qrt", [DIM, NQ], BF16, isOutput=False)
    wqkv = nc.declare_dram_parameter("wqkv", [DIM, 3 * DIM], BF16, isOutput=False)
    wproj = nc.declare_dram_parameter("wproj", [DIM, DIM], BF16, isOutput=False)
    maskT = nc.declare_dram_parameter("maskt", [N, NQ], BF16, isOutput=False)
    out = nc.declare_dram_parameter("out", [NQ, DIM], BF16, isOutput=True)

    Exp = mybir.ActivationFunctionType.Exp

    with tile.TileContext(nc) as tc, ExitStack() as ctx:
        persist = ctx.enter_context(tc.tile_pool(name="persist", bufs=1))
        bpool = ctx.enter_context(tc.tile_pool(name="bpool", bufs=1))
        bcpool = ctx.enter_context(tc.tile_pool(name="bcpool", bufs=2))
        esbp = ctx.enter_context(tc.tile_pool(name="esbp", bufs=3))
        small = ctx.enter_context(tc.tile_pool(name="small", bufs=2))
        bcast_pool = ctx.enter_context(tc.tile_pool(name="bcastp", bufs=1))
        outp = ctx.enter_context(tc.tile_pool(name="outp", bufs=2))

        # ---- persistent loads -------------------------------------------
        wq_sb = []
        for k in range(4):
            t = persist.tile([128, 3 * DIM], BF16, tag=f"wqkv{k}", name=f"wq{k}")
            nc.sync.dma_start(out=t, in_=wqkv[k * 128:(k + 1) * 128, :])
            wq_sb.append(t)
        wp_sb = []
        for k in range(4):
            t = persist.tile([128, DIM], BF16, tag=f"wproj{k}", name=f"wp{k}")
            nc.sync.dma_start(out=t, in_=wproj[k * 128:(k + 1) * 128, :])
            wp_sb.append(t)
        mask_sb = []
        for jb in range(JB):
            t = persist.tile([128, NQ], BF16, tag=f"mask{jb}", name=f"mask{jb}")
            nc.sync.dma_start(out=t, in_=maskT[jb * 128:(jb + 1) * 128, :])
            mask_sb.append(t)
        qr_sb = []
        for k in range(4):
            t = bpool.tile([128, NQ], BF16, tag=f"qrT{k}", name=f"qr{k}")
            nc.sync.dma_start(out=t, in_=qrT[k * 128:(k + 1) * 128, :])
            qr_sb.append(t)

        kt_sb = [bpool.tile([128, N], BF16, tag=f"kt{g}", name=f"kt{g}")
                 for g in range(4)]
        qt_sb = [bpool.tile([128, NQ], BF16, tag=f"qt{g}", name=f"qt{g}")
                 for g in range(4)]
        v_sb = [bpool.tile([128, H * (D + 1)], BF16, tag=f"v{jb}", name=f"v{jb}")
                for jb in range(JB)]
        pre_sb = [bpool.tile([128, NQ], BF16, tag=f"pre{g}", name=f"pre{g}")
                  for g in range(4)]

        # ---- dense-QKV helpers (run deferred, interleaved) --------------
        bc_map = {}  # jc -> [4 tiles of batT columns]

        def dma_bc(jc):
            tiles = [bcpool.tile([128, 512], BF16, tag=f"bc{k}", name=f"bc{k}_{jc}")
                     for k in range(4)]
            for k in range(4):
                nc.sync.dma_start(
                    out=tiles[k], in_=batT[k * 128:(k + 1) * 128, jc * 512:(jc + 1) * 512])
            bc_map[jc] = tiles

        def make_kt_chunk(pool, g, jc):
            ps = pool.tile([128, 512], F32, tag="xps")
            for k in range(4):
                nc.tensor.matmul(
                    ps,
                    wq_sb[k][:, DIM + 128 * g: DIM + 128 * g + 128],
                    bc_map[jc][k],
                    start=(k == 0), stop=(k == 3),
                )
            nc.vector.tensor_copy(kt_sb[g][:, jc * 512:(jc + 1) * 512], ps)

        def make_qt_half(pool, g, half):
            ps = pool.tile([128, 512], F32, tag="xps")
            for k in range(4):
                nc.tensor.matmul(
                    ps,
                    wq_sb[k][:, 128 * g: 128 * g + 128],
                    qr_sb[k][:, half * 512:(half + 1) * 512],
                    start=(k == 0), stop=(k == 3),
                )
            nc.vector.tensor_copy(qt_sb[g][:, half * 512:(half + 1) * 512], ps)

        def make_v(pool, jb):
            jc, nb = jb // 4, jb % 4
            ps = pool.tile([128, 512], F32, tag="xps")
            for k in range(4):
                nc.tensor.matmul(
                    ps,
                    bc_map[jc][k][:, nb * 128:(nb + 1) * 128],
                    wq_sb[k][:, 2 * DIM: 3 * DIM],
                    start=(k == 0), stop=(k == 3),
                )
            t = v_sb[jb]
            dst = bass.AP(
                tensor=t.tensor, offset=t.offset,
                ap=[t.ap[0], [D + 1, H], [1, D]],
            )
            nc.scalar.copy(dst, ps)
            ones = bass.AP(
                tensor=t.tensor, offset=t.offset + D,
                ap=[t.ap[0], [D + 1, H]],
            )
            nc.vector.memset(ones, 1.0)

        with (tc.tile_pool(name="dense", bufs=2, space="PSUM") as dense_ps,
              tc.tile_pool(name="stp", bufs=1, space="PSUM") as st_ps,
              tc.tile_pool(name="avp", bufs=1, space="PSUM") as av_ps):

            # ---- phase A: first two column chunks + first K chunk/Q ------
            dma_bc(0)
            dma_bc(1)
            for jc in range(2):
                make_kt_chunk(dense_ps, 0, jc)
            for half in range(2):
                make_qt_half(dense_ps, 0, half)
            for jb in range(8):
                make_v(dense_ps, jb)

            # ---- deferred dense work queues per head-pair group ----------
            def work_hh0():
                # DMA for chunk jc is queued one block ahead of its consumers
                u = [lambda: dma_bc(2), lambda: dma_bc(3)]
                for jc in range(2, JC):
                    if jc + 2 < JC:
                        u.append(lambda jc=jc: dma_bc(jc + 2))
                    u.append(lambda jc=jc: make_kt_chunk(dense_ps, 0, jc))
                    for nb in range(4):
                        u.append(lambda jb=4 * jc + nb: make_v(dense_ps, jb))
                return u

            def work_kt(g):
                u = [lambda: dma_bc(0), lambda: dma_bc(1)]
                for jc in range(JC):
                    if jc + 2 < JC:
                        u.append(lambda jc=jc: dma_bc(jc + 2))
                    u.append(lambda jc=jc, g=g: make_kt_chunk(dense_ps, g, jc))
                u.append(lambda g=g: make_qt_half(dense_ps, g, 0))
                u.append(lambda g=g: make_qt_half(dense_ps, g, 1))
                return u

            WORK = {hh: [] for hh in range(8)}
            WORK[0] = work_hh0()
            WORK[1] = work_kt(1)
            WORK[3] = work_kt(2)
            WORK[5] = work_kt(3)

            # ---- attention ----------------------------------------------
            def normalize_units(hh, av):
                # split into small closures so the work spreads over the
                # next group's early iterations; all av reads are PSUM
                # (all-SBUF tensor_tensor requires equal base partitions)
                state = {}

                def mkrs(hi):
                    rs = small.tile([1, NQ], F32, tag="rs")
                    nc.vector.tensor_copy(rs, av[64 * hi + 32: 64 * hi + 33, :])
                    state[hi] = rs

                def mkbc(hi):
                    rcp = small.tile([1, NQ], F32, tag="rcp")
                    nc.vector.reciprocal_approx_fast(rcp, state[hi])
                    rcpb = bcast_pool.tile([32, NQ], F32, tag="rcpb")
                    nc.gpsimd.partition_broadcast(rcpb, rcp[0:1, :], channels=32)
                    state[(hi, "b")] = rcpb

                def mkmul(hi):
                    h = 2 * hh + hi
                    g, band = h // 4, 32 * (h % 4)
                    nc.vector.tensor_mul(
                        pre_sb[g][band: band + 32, :],
                        av[64 * hi: 64 * hi + 32, :],
                        state[(hi, "b")],
                    )

                return [lambda: mkrs(0), lambda: mkrs(1),
                        lambda: mkbc(0), lambda: mkmul(0),
                        lambda: mkbc(1), lambda: mkmul(1)]

            for hh in range(8):
                av = av_ps.tile([128, NQ], F32, tag="av")
                work = list(WORK[hh])
                nw = len(work)
                WIN = JB - 4  # finish deferred work 4 iterations early (JIT lead)
                g = (2 * hh) // 4
                band = [32 * ((2 * hh + hi) % 4) for hi in range(2)]
                for jb in range(JB):
                    lo = nw * min(jb, WIN) // WIN
                    hi_ = nw * min(jb + 1, WIN) // WIN
                    for wi in range(lo, hi_):
                        work[wi]()
                    # st tile per i-half holds BOTH heads side by side, so
                    # adjacent matmuls sit at different PE row bands and
                    # column banks -> they stream concurrently on the array
                    sts = []
                    for ih in range(2):
                        st = st_ps.tile([128, NQ], F32, tag=f"st{ih}")
                        for hi in range(2):
                            nc.tensor.matmul(
                                st[:, hi * 512:(hi + 1) * 512],
                                kt_sb[g][band[hi]: band[hi] + 32, jb * 128:(jb + 1) * 128],
                                qt_sb[g][band[hi]: band[hi] + 32, ih * 512:(ih + 1) * 512],
                                start=True, stop=True,
                                tile_position=(band[hi], 0),
                            )
                        sts.append(st)
                    es = []
                    for ih in range(2):
                        e = esbp.tile([128, NQ], BF16, tag="e")
                        m = mask_sb[jb]
                        mrep = bass.AP(
                            tensor=m.tensor, offset=m.offset + ih * 512,
                            ap=[m.ap[0], [0, 2], [1, 512]],
                        )
                        idx = ((hh * JB + jb) * 2 + ih) % 16
                        if idx < 4:
                            # Schraudolph exp on DVE: bf16 bits via int16
                            nc.vector.tensor_scalar(
                                out=e.bitcast(I16),
                                in0=sts[ih],
                                scalar1=SCH_A, scalar2=SCH_B,
                                op0=AluOpType.mult, op1=AluOpType.add,
                            )
                            nc.vector.tensor_mul(e, e, mrep)
                        elif idx < 10:
                            nc.scalar.activation(e, sts[ih], Exp, scale=SCALE)
                            nc.gpsimd.tensor_mul(e, e, mrep)
                        else:
                            nc.scalar.activation(e, sts[ih], Exp, scale=SCALE)
                            nc.vector.tensor_mul(e, e, mrep)
                        es.append(e)
                    for ih in range(2):
                        for hi in range(2):
                            h = 2 * hh + hi
                            nc.tensor.matmul(
                                av[64 * hi: 64 * hi + 33, ih * 512:(ih + 1) * 512],
                                v_sb[jb][:, (D + 1) * h: (D + 1) * h + (D + 1)],
                                es[ih][:, hi * 512:(hi + 1) * 512],
                                start=(jb == 0), stop=(jb == JB - 1),
                                tile_position=(0, 64 * hi),
                                skip_group_check=True,
                            )
                # normalize at group end (av bufs=1: must complete before
                # the next group's first AV matmul reuses the PSUM buffer)
                for f in normalize_units(hh, av):
                    f()

            # ---- output projection --------------------------------------
            for ib in range(NQ // 128):
                ps = dense_ps.tile([128, DIM], F32, tag="xps")
                for g in range(4):
                    nc.tensor.matmul(
                        ps,
                        pre_sb[g][:, ib * 128:(ib + 1) * 128],
                        wp_sb[g],
                        start=(g == 0), stop=(g == 3),
                    )
                o = outp.tile([128, DIM], BF16, tag="o")
                nc.vector.tensor_copy(o, ps)
                nc.sync.dma_start(out=out[ib * 128:(ib + 1) * 128, :], in_=o)

    nc.compile()
    return nc


def _prep_inputs(batch, w_qkv, w_proj, custom_mask):
    batch = np.asarray(batch, np.float32)
    wqkv_bf = np.asarray(w_qkv, np.float32).astype(NPBF16)
    wproj_bf = np.asarray(w_proj, np.float32).astype(NPBF16)
    m = np.asarray(custom_mask, np.float32)[0, 0]  # [N, N] 0/1
    in_maps = []
    for c in range(NCORES):
        b, q = c // 4, c % 4
        rows = slice(q * NQ, (q + 1) * NQ)
        batT = np.ascontiguousarray(batch[b].T).astype(NPBF16)
        qrT = np.ascontiguousarray(batch[b, rows, :].T).astype(NPBF16)
        mT = np.ascontiguousarray(m[rows, :].T).astype(NPBF16)
        in_maps.append({
            "batt": batT, "qrt": qrT, "wqkv": wqkv_bf,
            "wproj": wproj_bf, "maskt": mT,
        })
    return in_maps


def _run(in_maps, trace=False, **kw):
    if "nc" not in _CACHE:
        _CACHE["nc"] = build_nc()
    return run_bass_kernel_spmd(
        _CACHE["nc"], in_maps, core_ids=list(range(NCORES)), trace=trace, **kw
    )


def kernel(batch, w_qkv, w_proj, custom_mask):
    in_maps = _prep_inputs(batch, w_qkv, w_proj, custom_mask)
    res = _run(in_maps)
    full = np.empty((B, N, DIM), np.float32)
    for c in range(NCORES):
        b, q = c // 4, c % 4
        full[b, q * NQ:(q + 1) * NQ, :] = res.results[c]["out"].astype(np.float32)
    return full
